# revision 7
# baseline (speedup 1.0000x reference)
"""BertAdapter (TT-decomposed bottleneck MLP) Trainium2 kernel, bf16 edition.

Computes  out = x + gelu(x @ W_down + b_down) @ W_up + b_up  where the
adapter weights arrive as tensor-train cores.  The TT cores are tiny
(~50K params), so they are contracted to dense matrices on the host and
the device kernel runs the dense bottleneck MLP data-parallel across
8 NeuronCores (4096 tokens per core).

The kernel is HBM-bandwidth/PE-bound, so the data plane runs in bf16
(rel err ~5e-3, tolerance 2e-2): x is cast to bf16 on the host, all
HBM traffic and matmul operands are bf16 (f32 PSUM accumulation), and
the bf16 output is upcast on the host.  This halves HBM bytes and
runs transposes at 1 PE-cycle/row.

Per-core device kernel, per 2048-token superblock (one input DMA,
alternating between the SP and ACT HWDGE rings; one gpsimd output DMA):
  4 compute passes of 512 tokens each:
  1. PE-transpose x -> xT (hidden on partitions), PSUM->SBUF copies on
     the scalar engine (1 of 3 on vector).
  2. Down-proj: 6 accumulating matmuls W_down_chunk.T @ xT_chunk ->
     PSUM [65, 512].
  3. Exact-erf Gelu + b_down bias on the scalar engine; row 64
     evaluates gelu(gelu^-1(1)) = 1, producing the ones-row that
     multiplies the b_up row of wub in the up-projection.
  4. Up-proj: act serves as the stationary operand, output lands in
     natural [token, hidden] layout with b_up folded in.
  5. Residual add (x + up) on the vector engine into the out tile.
"""

import os
import sys
from contextlib import ExitStack

import numpy as np

for _p in ("/opt/trn_rl_repo", "/root/.axon_site/_ro/trn_rl_repo"):
    if os.path.isdir(_p) and _p not in sys.path:
        sys.path.insert(0, _p)

import ml_dtypes

import concourse.bass as bass
import concourse.tile as tile
from concourse import mybir
from concourse.bass_utils import run_bass_kernel_spmd
from concourse.masks import make_identity

P = 128                 # SBUF partitions
H = 768                 # hidden size
A = 64                  # adapter bottleneck size
B, S = 16, 2048         # full batch / seq
NCORES = 8
TOK = (B // NCORES) * S  # tokens per core = 4096
SBLK = 2048             # tokens per DMA superblock
TBLK = 512              # tokens per compute pass
IPB = SBLK // P         # 16 token-subtiles per superblock
TSUB = TBLK // P        # 4 subtiles per compute pass
NPASS = SBLK // TBLK    # 4 compute passes per superblock
HC = H // P             # hidden chunks of 128
F32 = mybir.dt.float32
BF16 = mybir.dt.bfloat16
MMDT = mybir.dt.float32r  # kept for importers of the old constant
NPBF = ml_dtypes.bfloat16


def _legalize_waits(nc):
    """Split multi-wait instructions for this walrus build.

    The walrus in this toolchain accepts only ONE sync-wait per
    instruction ("Too many sync wait commands" in setupSyncWait), while
    Tile freely attaches several.  Hoist all but the last wait of each
    instruction onto freshly inserted same-engine NoOps directly before
    it — engine program order makes this semantically identical.
    """
    n = 0

    def fix_block(bb):
        nonlocal n
        insts = bb.instructions
        i = 0
        while i < len(insts):
            inst = insts[i]
            for sub in getattr(inst, "blocks", None) or []:
                fix_block(sub)
            si = inst.sync_info
            waits = list(si.on_wait) if si and si.on_wait else []
            if len(waits) > 1:
                for w in waits[:-1]:
                    nop = mybir.InstNoOp(name=f"I-waitsplit-{n}", ins=[], outs=[])
                    n += 1
                    nop.engine = inst.engine
                    nop.sync_info = mybir.SyncInfo(on_wait=[w], on_update=[])
                    insts.insert(i, nop)
                    i += 1
                inst.sync_info = mybir.SyncInfo(
                    on_wait=[waits[-1]], on_update=list(si.on_update)
                )
            i += 1

    for fn in nc.m.functions:
        for bb in fn.blocks:
            fix_block(bb)
    return nc


def build_nc(tok=TOK, repeats=1):
    nsb = tok // SBLK
    nc = bass.Bass("TRN2", target_bir_lowering=False, debug=False)
    x = nc.dram_tensor("x", [tok, H], BF16, kind="ExternalInput").ap()
    # wd/bd carry an extra adapter column: wd col A is zeros and bd[A] is
    # gelu^-1(1.0), so the gelu writes a constant ones-row into act[A] that
    # multiplies the b_up row of wub in the up-projection (bias via matmul).
    wd = nc.dram_tensor("wd", [H, A + 1], F32, kind="ExternalInput").ap()
    wub = nc.dram_tensor("wub", [A + 1, H], F32, kind="ExternalInput").ap()
    bd = nc.dram_tensor("bd", [A + 1, 1], F32, kind="ExternalInput").ap()
    y = nc.dram_tensor("y", [tok, H], BF16, kind="ExternalOutput").ap()

    with ExitStack() as ctx:
        tc = ctx.enter_context(tile.TileContext(nc))
        const = ctx.enter_context(tc.tile_pool(name="const", bufs=1))
        xin = ctx.enter_context(tc.tile_pool(name="xin", bufs=3))
        xtp = ctx.enter_context(tc.tile_pool(name="xt", bufs=2))
        actp = ctx.enter_context(tc.tile_pool(name="act", bufs=2))
        outp = ctx.enter_context(tc.tile_pool(name="out", bufs=2))
        ps_t = ctx.enter_context(tc.tile_pool(name="ps_t", bufs=2, space="PSUM"))
        ps_d = ctx.enter_context(tc.tile_pool(name="ps_d", bufs=2, space="PSUM"))
        ps_u = ctx.enter_context(tc.tile_pool(name="ps_u", bufs=2, space="PSUM"))

        # bf16 identity built via f32 affine_select + engine cast (bf16
        # affine_select crashes the exec unit)
        ident_st = const.tile([P, P], F32)
        make_identity(nc, ident_st)
        ident = const.tile([P, P], BF16)
        nc.vector.tensor_copy(ident[:], ident_st[:])
        # weights: DMA to fp32 staging, engine-copy (cast) to bf16
        wd_st = const.tile([P, HC, A + 1], F32)
        nc.sync.dma_start(wd_st[:], wd.rearrange("(c p) a -> p c a", p=P))
        wub_st = const.tile([A + 1, H], F32)
        nc.sync.dma_start(wub_st[:], wub[:])
        wd_sb = const.tile([P, HC, A + 1], BF16)
        wub_sb = const.tile([A + 1, H], BF16)
        nc.vector.tensor_copy(wd_sb[:], wd_st[:])
        nc.vector.tensor_copy(wub_sb[:], wub_st[:])
        bd_sb = const.tile([A + 1, 1], F32)
        nc.sync.dma_start(bd_sb[:], bd[:])
        # touch the Gelu table up front so its ACT_TABLE_LOAD overlaps the
        # first input DMA instead of stalling the first block
        warm = const.tile([1, 1], F32)
        nc.scalar.activation(
            warm[:], ident_st[0:1, 0:1], mybir.ActivationFunctionType.Gelu
        )

        # superblock views: [p, i, h] = x[b*SBLK + i*128 + p, h]
        x_blk = x.rearrange("(b i p) h -> b p i h", i=IPB, p=P)
        y_blk = y.rearrange("(b i p) h -> b p i h", i=IPB, p=P)

        pending_up = None
        for it in range(nsb * repeats):
            b = it % nsb
            # one DMA per 2048-token superblock on the SP HWDGE ring
            xt_in = xin.tile([P, IPB, H], BF16, tag="xin")
            nc.sync.dma_start(xt_in[:], x_blk[b])

            ot = outp.tile([P, IPB, H], BF16, tag="ot")
            for hp in range(NPASS):
                xs = [xt_in[:, hp * TSUB + i, :] for i in range(TSUB)]
                # transpose x -> xT (hidden on partitions)
                xt_sb = xtp.tile([P, HC, TBLK], BF16, tag="xt")
                for j in range(HC):
                    pt = ps_t.tile([P, TBLK], BF16)
                    for i in range(TSUB):
                        nc.tensor.transpose(
                            pt[:, i * P : (i + 1) * P],
                            xs[i][:, j * P : (j + 1) * P],
                            ident[:],
                        )
                    if j % 3 == 2:
                        nc.vector.tensor_copy(xt_sb[:, j, :], pt[:])
                    else:
                        nc.scalar.copy(xt_sb[:, j, :], pt[:])
                # deferred up-projection of the previous pass: PE runs it
                # after this pass's transposes, so it never stalls on gelu
                if pending_up is not None:
                    pending_up()
                    pending_up = None
                # down projection: accumulate over hidden chunks
                pd = ps_d.tile([A + 1, TBLK], F32)
                for j in range(HC):
                    nc.tensor.matmul(
                        pd[:], wd_sb[:, j, :], xt_sb[:, j, :],
                        start=(j == 0), stop=(j == HC - 1),
                    )
                # exact-erf gelu with per-partition b_down bias; row A
                # computes gelu(0 + gelu^-1(1)) = 1.0, the b_up multiplier
                act = actp.tile([A + 1, TBLK], BF16, tag="act")
                nc.scalar.activation(
                    act[:], pd[:], mybir.ActivationFunctionType.Gelu,
                    bias=bd_sb[:, 0:1],
                )

                # up projection back to [token, hidden] layout + residual,
                # deferred one pass; the last pass also emits the out-DMA
                def mk_up(act=act, xs=xs, ot=ot, hp=hp, b=b,
                          last=(hp == NPASS - 1)):
                    def emit():
                        for i in range(TSUB):
                            pu = ps_u.tile([P, H], F32)
                            lhsT = act[:, i * P : (i + 1) * P]
                            nc.tensor.matmul(
                                pu[:, 0:512], lhsT, wub_sb[:, 0:512],
                                start=True, stop=True,
                            )
                            nc.tensor.matmul(
                                pu[:, 512:H], lhsT, wub_sb[:, 512:H],
                                start=True, stop=True,
                            )
                            nc.vector.tensor_add(
                                ot[:, hp * TSUB + i, :], xs[i][:], pu[:]
                            )
                        # outputs leave via the (otherwise idle) GPSIMD
                        # SWDGE path, one 512-token slice per pass as soon
                        # as its residual adds land: the out stream drains
                        # immediately and interleaves with the input stream
                        # instead of bursting once per superblock
                        h0 = hp * TSUB
                        nc.gpsimd.dma_start(
                            y_blk[b][:, h0 : h0 + TSUB, :],
                            ot[:, h0 : h0 + TSUB, :],
                        )
                    return emit

                pending_up = mk_up()
        if pending_up is not None:
            pending_up()
    return _legalize_waits(nc)


def _tt_to_matrix(cores, in_dim, out_dim):
    t = cores[0]
    for c in cores[1:]:
        t = np.tensordot(t, c, axes=([-1], [0]))
    t = np.squeeze(t, axis=(0, -1))
    return np.ascontiguousarray(t.reshape(in_dim, out_dim).astype(np.float32))


def _gelu_inv_one():
    """x with x * Phi(x) == 1 (erf gelu), solved by Newton in float64."""
    import math

    def gelu(x):
        return x * 0.5 * (1.0 + math.erf(x / math.sqrt(2.0)))

    def dgelu(x):
        return 0.5 * (1.0 + math.erf(x / math.sqrt(2.0))) + x * math.exp(
            -0.5 * x * x
        ) / math.sqrt(2.0 * math.pi)

    x = 1.15
    for _ in range(40):
        x -= (gelu(x) - 1.0) / dgelu(x)
    return x


_NC_CACHE = {}


def _get_nc(tok=TOK):
    if tok not in _NC_CACHE:
        _NC_CACHE[tok] = build_nc(tok)
    return _NC_CACHE[tok]


def kernel(hidden_states, d0, d1, d2, d3, d4, u0, u1, u2, u3, u4,
           b_down, b_up, **_run_kwargs):
    hs = np.asarray(hidden_states, dtype=np.float32)
    w_down = _tt_to_matrix(
        [np.asarray(c, np.float32) for c in (d0, d1, d2, d3, d4)], H, A
    )
    w_up = _tt_to_matrix(
        [np.asarray(c, np.float32) for c in (u0, u1, u2, u3, u4)], A, H
    )
    wd = np.concatenate([w_down, np.zeros((H, 1), np.float32)], axis=1)
    wd = np.ascontiguousarray(wd)
    wub = np.ascontiguousarray(
        np.concatenate([w_up, np.asarray(b_up, np.float32)[None, :]], axis=0)
    )
    bd = np.concatenate(
        [
            np.asarray(b_down, np.float32).reshape(A, 1),
            np.full((1, 1), _gelu_inv_one(), np.float32),
        ],
        axis=0,
    )
    bd = np.ascontiguousarray(bd)

    flat = hs.reshape(B * S, H).astype(NPBF)
    in_maps = [
        {
            "x": np.ascontiguousarray(flat[c * TOK : (c + 1) * TOK]),
            "wd": wd,
            "wub": wub,
            "bd": bd,
        }
        for c in range(NCORES)
    ]
    nc = _get_nc()
    res = run_bass_kernel_spmd(nc, in_maps, list(range(NCORES)), **_run_kwargs)
    out = np.concatenate(
        [res.results[c]["y"].astype(np.float32) for c in range(NCORES)], axis=0
    )
    out = out.reshape(B, S, H)
    if _run_kwargs:
        kernel.last_results = res
    return out
